# revision 27
# baseline (speedup 1.0000x reference)
"""Trainium2 Bass kernel for nn_Attention_29661044146348.

Diffusion-style attention block: GroupNorm(32) -> 1x1-conv qkv -> single-head
attention over h*w positions (d = C = 512) -> 1x1-conv out -> residual.
Input x is [8, 512, 64, 64]; batch is data-parallel across the 8 NeuronCores
(one batch element per core), no collectives.

Per-core layout strategy ("S^T layout" flash attention, zero transposes in the
hot path):
  - scores are computed transposed, S^T[j, i] (keys on partitions), via
    lhsT = K^T chunks, rhs = Q^T block -- both natural outputs of the qkv
    matmul.
  - all hot-path matmuls run in fp8e4 with perf_mode=DoubleRow (two
    128-contraction planes per pass, ~1.5x bf16 PE rate). Value ranges are
    pre-scaled into fp8e4's safe window (TRN e4m3 max-normal is 240):
      * weights are stored as fp8(16*W); projections un-scale by 1/16 when
        leaving PSUM.
      * x is normalized on the fly: xn8 = fp8(A*x + B) via one scalar-engine
        activation per channel tile (A, B are the folded GroupNorm per-channel
        scale/bias, computed per 128-channel tile so the pipeline starts as
        soon as that tile's stats are in).
      * P = exp(scale*S^T - 2): the -2 shift keeps the max ~e^3.5 << 240 and
        cancels exactly in the softmax normalization.
      * O is normalized with rbc = 64/denom so fp8 O sits at ~N(0, 1.7),
        clear of the subnormal floor; the final projection un-scales by
        1/(16*64).
  - softmax denominators: bf16 elementwise accumulation on DVE, one
    (1/64)*ones[128x128] matmul for the cross-partition reduce+broadcast,
    and a DVE reciprocal for rbc.
  - the attention i-block loop is software-pipelined: block n-1's PV /
    denominator / output-projection matmuls are interleaved into block n's
    QK issue stream so the PE never waits for the scalar engine's exp.
  - the residual is streamed back from DRAM in fp32 per block (cheaper than
    holding a bf16 copy of x in SBUF and frees the GpSimd/DVE shared port).
  - engine balance: Scalar does xn8 + exp + K bias + half the V copies +
    final bias; DVE does denom adds + Q bias + other half of V + O normalize
    + residual; PE does matmuls + transposes only.
"""

import jax
import numpy as np
from jax.experimental.shard_map import shard_map
from jax.sharding import Mesh, NamedSharding, PartitionSpec

import bass_rust
import concourse.bass as bass
import concourse.tile as tile
from concourse import bass2jax, mybir
from concourse.alu_op_type import AluOpType
from concourse.masks import make_identity

F32 = mybir.dt.float32
BF16 = mybir.dt.bfloat16
F8 = mybir.dt.float8e4
DR = mybir.MatmulPerfMode.DoubleRow
IDENT_F = mybir.ActivationFunctionType.Identity

C = 512          # channels == attention dim
NT = C // 128    # channel tiles (4)
NC2 = NT // 2    # DoubleRow channel-tile pairs (2)
GROUPS = 32
EPS = 1e-5
ATT_SCALE = float(C) ** -0.5
IB = 512         # attention i-block (queries per block)
ESHIFT = 2.0     # exp(scale*s - ESHIFT); cancels in softmax normalization
WSCALE = 16.0    # fp8 weights stored as 16*W
OSCALE = 64.0    # fp8 O stored as 64*O/denom


def _split_multi_waits(nc):
    """The staged walrus build rejects >1 sync-wait per instruction; hoist
    extra waits onto single-wait NOPs placed immediately before."""
    ctr = 0
    for bb in nc.main_func.blocks:
        insts = bb.instructions
        i = 0
        while i < len(insts):
            ins = insts[i]
            si = ins.sync_info
            if si is not None:
                waits = list(si.on_wait)
                if len(waits) > 1:
                    si.on_wait = waits[-1:]
                    for w in waits[:-1]:
                        nop = mybir.InstNoOp(name=f"waitsplit-{ctr}", ins=[], outs=[])
                        ctr += 1
                        nop.engine = ins.engine
                        nop.sync_info = bass_rust.SyncInfo(on_wait=[w], on_update=[])
                        nc.register_instruction(nop, overwrite=True)
                        insts.insert(i, nop)
                        i += 1
            i += 1
    return ctr


def build_nc(S):
    S8 = S // 512     # seq chunks of 512
    JT = S // 128     # attention key chunks
    NIB = S // IB     # attention query blocks

    nc = bass.Bass()
    x_ext = nc.declare_dram_parameter("x", [C, S], F32, isOutput=False)
    gnw_ext = nc.declare_dram_parameter("gn_weight", [C], F32, isOutput=False)
    gnb_ext = nc.declare_dram_parameter("gn_bias", [C], F32, isOutput=False)
    qkvw_ext = nc.declare_dram_parameter("qkv_w", [3 * C, C], F32, isOutput=False)
    qkvb_ext = nc.declare_dram_parameter("qkv_b", [3 * C], F32, isOutput=False)
    outw_ext = nc.declare_dram_parameter("out_w", [C, C], F32, isOutput=False)
    outb_ext = nc.declare_dram_parameter("out_b", [C], F32, isOutput=False)
    out_ext = nc.declare_dram_parameter("out", [C, S], F32, isOutput=True)

    ov = out_ext[:].rearrange("(t p) s -> p t s", p=128)
    xv = x_ext[:].rearrange("(t p) s -> p t s", p=128)

    with tile.TileContext(nc) as tc:
        with (
            tc.tile_pool(name="consts", bufs=1) as consts,
            tc.tile_pool(name="big", bufs=1) as big,
            tc.tile_pool(name="gn_small", bufs=1) as gn_small,
        ):
            # ---------------- on-chip constants (no DMA) ----------------
            ident = consts.tile([128, 128], F32)
            make_identity(nc, ident)
            # 1/64 so the denominator reduce+broadcast matmul lands D/OSCALE
            # in PSUM; the plain reciprocal then directly yields OSCALE/D.
            onesbb = consts.tile([128, 128], BF16)
            nc.vector.memset(onesbb, 1.0 / OSCALE)
            ind = consts.tile([128, 8], F32)       # ind[p,g] = (p//16 == g)
            nc.vector.memset(ind, 1.0)
            nc.gpsimd.affine_select(
                out=ind, in_=ind, compare_op=mybir.AluOpType.is_ge, fill=0.0,
                base=0, pattern=[[-16, 8]], channel_multiplier=1)
            nc.gpsimd.affine_select(
                out=ind, in_=ind, compare_op=mybir.AluOpType.is_ge, fill=0.0,
                base=15, pattern=[[16, 8]], channel_multiplier=-1)
            indT = consts.tile([8, 128], F32)
            nc.vector.memset(indT, 1.0)
            nc.gpsimd.affine_select(
                out=indT, in_=indT, compare_op=mybir.AluOpType.is_ge, fill=0.0,
                base=0, pattern=[[1, 128]], channel_multiplier=-16)
            nc.gpsimd.affine_select(
                out=indT, in_=indT, compare_op=mybir.AluOpType.is_ge, fill=0.0,
                base=15, pattern=[[-1, 128]], channel_multiplier=16)
            eps8 = consts.tile([8, 1], F32)
            nc.vector.memset(eps8, EPS)
            negshift = consts.tile([128, 1], F32)
            nc.vector.memset(negshift, -ESHIFT)

            # resident big tiles (all fp8: 72KB/partition total)
            xn8 = big.tile([128, NT, S], F8)         # fp8(A*x + B) qkv input
            wT8 = big.tile([128, NT, 3 * C], F8)     # fp8(16 * qkv_w^T)
            owT8 = big.tile([128, NT, C], F8)        # fp8(16 * out_w^T)
            kT8 = big.tile([128, NT, S], F8)         # K^T  [c, s]
            q8 = big.tile([128, NT, S], F8)          # Q^T  [c, s] (resident)
            Vt8 = big.tile([128, JT, C], F8)         # V    [s, c] by key chunk

            qkvb12 = consts.tile([128, 3 * NT], F32)
            wv = consts.tile([128, NT], F32)
            bv = consts.tile([128, NT], F32)
            obt = consts.tile([128, NT], F32)
            ob_eff = consts.tile([128, NT], F32)
            gAt = gn_small.tile([128, NT], F32)      # per-channel GN scale
            gBt = gn_small.tile([128, NT], F32)      # per-channel GN bias
            stAll = gn_small.tile([128, NT, S8, 6], F32)
            vbr8 = gn_small.tile([128, NT, 2], F8)

            qb = qkvb12[:, 0:NT]
            kb = qkvb12[:, NT:2 * NT]

            # ------- startup: x stats + on-the-fly normalization ------------
            # Engine queues are kept decoupled: DVE runs stats + tiny combine
            # ops only (per channel tile, so tile t's xn8 can start as soon as
            # its own stats are in); Scalar runs the xn8 casts; the PE runs
            # the tiny combine matmuls, then weight transposes.
            with (
                tc.tile_pool(name="wnat", bufs=16) as wnat,
                tc.tile_pool(name="xstream", bufs=1) as xstream,
                tc.tile_pool(name="stp", bufs=2) as stp,
                tc.tile_pool(name="pst", bufs=3, space="PSUM") as pst,
                tc.tile_pool(name="psg", bufs=2, space="PSUM") as psg,
            ):
                # Weight DMAs lead: their transposes + fp8 casts run on the
                # PE/DVE/Scalar while the (larger) x stream is still in
                # flight. x follows as 4 chunk-DMAs per channel tile, t-major
                # so earlier tiles complete earlier and the per-tile
                # stats -> combine -> xn8 chains pipeline against the stream.
                wn_tiles = []
                for r in range(3 * C // 128):
                    wn = wnat.tile([128, C], F32)
                    nc.sync.dma_start(out=wn[:], in_=qkvw_ext[r * 128:(r + 1) * 128, :])
                    wn_tiles.append(wn)
                for r in range(C // 128):
                    wn = wnat.tile([128, C], F32)
                    nc.sync.dma_start(out=wn[:], in_=outw_ext[r * 128:(r + 1) * 128, :])
                    wn_tiles.append(wn)
                nc.sync.dma_start(out=qkvb12[:], in_=qkvb_ext[:].rearrange("(t p) -> p t", p=128))
                nc.sync.dma_start(out=wv[:], in_=gnw_ext[:].rearrange("(t p) -> p t", p=128))
                nc.sync.dma_start(out=bv[:], in_=gnb_ext[:].rearrange("(t p) -> p t", p=128))
                nc.sync.dma_start(out=obt[:], in_=outb_ext[:].rearrange("(t p) -> p t", p=128))
                xcs = [xstream.tile([128, S], F32, name=f"xc{t}") for t in range(NT)]
                XH = S // 8
                for t in range(NT):
                    for h in range(8):
                        nc.sync.dma_start(
                            out=xcs[t][:, h * XH:(h + 1) * XH],
                            in_=x_ext[t * 128:(t + 1) * 128, h * XH:(h + 1) * XH])

                # weight transposes (PE) -> fp8(16*W) casts, alternating
                # DVE/Scalar, K rows first (they gate the first qkv matmul)
                def wemit(r, c4, dst):
                    psT = pst.tile([128, 128], F32)
                    nc.tensor.transpose(psT[:], wn_tiles[r][:, c4 * 128:(c4 + 1) * 128], ident[:])
                    if (r + c4) % 2 == 0:
                        nc.vector.tensor_scalar_mul(dst, psT[:], WSCALE)
                    else:
                        nc.scalar.activation(out=dst, in_=psT[:], func=IDENT_F,
                                             scale=WSCALE)

                for r in list(range(4, 8)) + list(range(4)) + list(range(8, 12)):
                    for c4 in range(NT):
                        wemit(r, c4, wT8[:, c4, r * 128:(r + 1) * 128])
                for r in range(C // 128):
                    for c4 in range(NT):
                        wemit(12 + r, c4, owT8[:, c4, r * 128:(r + 1) * 128])

                for t in range(NT):
                    xc = xcs[t]
                    for s8 in range(S8):
                        nc.vector.bn_stats(out=stAll[:, t, s8, :],
                                           in_=xc[:, s8 * 512:(s8 + 1) * 512])
                    # per-tile GroupNorm combine (groups are 16 consecutive
                    # channels, so a 128-channel tile is self-contained)
                    mvt = stp.tile([128, 2], F32)
                    nc.vector.bn_aggr(out=mvt[:], in_=stAll[:, t, :, :])
                    st2 = stp.tile([128, 2], F32)
                    nc.vector.tensor_copy(st2[:, 0:1], mvt[:, 0:1])
                    sqt = stp.tile([128, 1], F32)
                    nc.vector.tensor_mul(sqt[:], mvt[:, 0:1], mvt[:, 0:1])
                    nc.vector.tensor_add(st2[:, 1:2], mvt[:, 1:2], sqt[:])
                    psG = psg.tile([8, 2], F32, tag="psG")
                    nc.tensor.matmul(psG[:], ind[:], st2[:], start=True, stop=True)
                    gsb = stp.tile([8, 2], F32)
                    nc.vector.tensor_scalar_mul(gsb[:], psG[:], 1.0 / 16.0)
                    sq8 = stp.tile([8, 1], F32)
                    nc.vector.tensor_mul(sq8[:], gsb[:, 0:1], gsb[:, 0:1])
                    varr = stp.tile([8, 1], F32)
                    nc.vector.tensor_sub(varr[:], gsb[:, 1:2], sq8[:])
                    sd8 = stp.tile([8, 1], F32)
                    nc.scalar.activation(out=sd8[:], in_=varr[:],
                                         func=mybir.ActivationFunctionType.Sqrt,
                                         bias=eps8[:], scale=1.0)
                    nc.vector.reciprocal(gsb[:, 1:2], sd8[:])
                    psBC = psg.tile([128, 2], F32, tag="psBC")
                    nc.tensor.matmul(psBC[:], indT[:], gsb[:], start=True, stop=True)
                    nc.vector.tensor_mul(gAt[:, t:t + 1], psBC[:, 1:2], wv[:, t:t + 1])
                    tmp1 = stp.tile([128, 1], F32)
                    nc.vector.tensor_mul(tmp1[:], psBC[:, 0:1], gAt[:, t:t + 1])
                    nc.vector.tensor_sub(gBt[:, t:t + 1], bv[:, t:t + 1], tmp1[:])
                    # xn8 = fp8(A*x + B), one scalar op per tile, pipelined
                    # against the x stream tile by tile
                    nc.scalar.activation(out=xn8[:, t, :], in_=xc[:], func=IDENT_F,
                                         bias=gBt[:, t:t + 1], scale=gAt[:, t:t + 1])

                # effective out bias: out_b + out_w @ v_bias (att rows sum
                # to 1, so the v bias is folded past the attention)
                nc.vector.memset(vbr8[:], 0.0)
                for c4 in range(NT):
                    nc.vector.tensor_copy(vbr8[:, c4, 0:1], qkvb12[:, 2 * NT + c4:2 * NT + c4 + 1])
                for oc in range(NT):
                    psE = psg.tile([128, 2], F32, tag="psBC")
                    for c4 in range(NT):
                        nc.tensor.matmul(psE[:], owT8[:, c4, oc * 128:(oc + 1) * 128],
                                         vbr8[:, c4, :],
                                         start=(c4 == 0), stop=(c4 == NT - 1))
                    nc.vector.tensor_scalar(out=ob_eff[:, oc:oc + 1], in0=psE[:, 0:1],
                                            scalar1=1.0 / WSCALE,
                                            scalar2=obt[:, oc:oc + 1],
                                            op0=AluOpType.mult, op1=AluOpType.add)

            # ---------------- qkv projection (fp8 DoubleRow) ----------------
            with (
                tc.tile_pool(name="p8p", bufs=2) as p8p,
                tc.tile_pool(name="accp", bufs=2) as accp,
                tc.tile_pool(name="rbcp", bufs=2) as rbcp,
                tc.tile_pool(name="oTp", bufs=2) as oTp,
                tc.tile_pool(name="osbp", bufs=2) as osbp,
                tc.tile_pool(name="xrp", bufs=2) as xrp,
            ):
                with tc.tile_pool(name="psq", bufs=4, space="PSUM") as psq:
                    for s8 in range(S8):
                        sl = slice(s8 * 512, (s8 + 1) * 512)
                        # K^T (scalar applies bias + 1/16)
                        for o4 in range(NT):
                            psK = psq.tile([128, 512], F32, tag="psq")
                            for cc in range(NC2):
                                nc.tensor.matmul(psK[:],
                                                 wT8[:, 2 * cc:2 * cc + 2, C + o4 * 128:C + (o4 + 1) * 128],
                                                 xn8[:, 2 * cc:2 * cc + 2, sl],
                                                 start=(cc == 0), stop=(cc == NC2 - 1),
                                                 perf_mode=DR)
                            nc.scalar.activation(out=kT8[:, o4, sl], in_=psK[:],
                                                 func=IDENT_F,
                                                 bias=kb[:, o4:o4 + 1], scale=1.0 / WSCALE)
                        # Q^T (DVE applies bias + 1/16), resident in SBUF
                        for o4 in range(NT):
                            psQ = psq.tile([128, 512], F32, tag="psq")
                            for cc in range(NC2):
                                nc.tensor.matmul(psQ[:],
                                                 wT8[:, 2 * cc:2 * cc + 2, o4 * 128:(o4 + 1) * 128],
                                                 xn8[:, 2 * cc:2 * cc + 2, sl],
                                                 start=(cc == 0), stop=(cc == NC2 - 1),
                                                 perf_mode=DR)
                            nc.vector.tensor_scalar(out=q8[:, o4, sl], in0=psQ[:],
                                                    scalar1=1.0 / WSCALE,
                                                    scalar2=qb[:, o4:o4 + 1],
                                                    op0=AluOpType.mult, op1=AluOpType.add)
                        # V (keys on partitions): lhsT = xn chunk, rhs = w_v^T.
                        # The 1/16 un-scale is folded into the PV normalize
                        # (rbc), so V extraction is a plain copy, alternating
                        # Scalar/DVE to balance the engines.
                        for j4 in range(4):
                            psV = psq.tile([128, 512], F32, tag="psq")
                            for cc in range(NC2):
                                nc.tensor.matmul(psV[:],
                                                 xn8[:, 2 * cc:2 * cc + 2,
                                                     s8 * 512 + j4 * 128:s8 * 512 + (j4 + 1) * 128],
                                                 wT8[:, 2 * cc:2 * cc + 2, 2 * C:3 * C],
                                                 start=(cc == 0), stop=(cc == NC2 - 1),
                                                 perf_mode=DR)
                            if j4 % 2 == 0:
                                nc.scalar.copy(Vt8[:, s8 * 4 + j4, :], psV[:])
                            else:
                                nc.vector.tensor_copy(Vt8[:, s8 * 4 + j4, :], psV[:])

                # ---------------- attention + output projection -------------
                # Software-pipelined: block n-1's PV + denominator + output
                # projection are interleaved into block n's QK issue stream so
                # the PE never stalls on the scalar engine's exp.
                with (
                    tc.tile_pool(name="psS", bufs=4, space="PSUM") as psSp,
                    tc.tile_pool(name="psO", bufs=3, space="PSUM") as psOp,
                    tc.tile_pool(name="psB", bufs=1, space="PSUM") as psBp,
                ):
                    # V carries a stray 16x (weights were 16*W and V skipped
                    # the un-scale): fold 1/16 into the O normalization.
                    PV_UNSCALE = 1.0 / WSCALE

                    def make_tail(n, P8, acc):
                        il = slice(n * IB, (n + 1) * IB)
                        oT8 = oTp.tile([128, NT, IB], F8)
                        rbc = rbcp.tile([128, IB], F32)
                        osb = osbp.tile([128, NT, IB], F32)
                        xres = xrp.tile([128, NT, IB], F32)
                        nc.sync.dma_start(out=xres[:], in_=xv[:, :, il])
                        psos = {}
                        ops = []

                        # the reciprocal is split into 4 sub-ops so it never
                        # clogs the DVE queue (a solid 3.4us reciprocal stalls
                        # the denominator adds and, through them, the PE)
                        psBbox = {}

                        def op_denom():
                            psBbox[0] = psBp.tile([128, IB], F32, tag="psB",
                                                  name=f"psB_{n}")
                            nc.tensor.matmul(psBbox[0][:], onesbb[:], acc[:],
                                             start=True, stop=True)
                        ops.append(op_denom)
                        recip_ops = []
                        for k in range(4):
                            def op_recip(k=k):
                                nc.vector.reciprocal(
                                    out=rbc[:, k * 128:(k + 1) * 128],
                                    in_=psBbox[0][:, k * 128:(k + 1) * 128])
                            recip_ops.append(op_recip)

                        for c4 in range(NT):
                            for jj in range(JT // 2):
                                def op_pv(c4=c4, jj=jj):
                                    if jj == 0:
                                        psos[c4] = psOp.tile([128, IB], F32, tag="psO",
                                                             name=f"psO_{n}_{c4}")
                                    nc.tensor.matmul(psos[c4],
                                                     Vt8[:, 2 * jj:2 * jj + 2, c4 * 128:(c4 + 1) * 128],
                                                     P8[:, 2 * jj:2 * jj + 2, :],
                                                     start=(jj == 0), stop=(jj == JT // 2 - 1),
                                                     perf_mode=DR)
                                    if jj == JT // 2 - 1:
                                        # oT = (psO/16) * (64/D), folded: psO * rbc / 16
                                        nc.vector.scalar_tensor_tensor(
                                            out=oT8[:, c4, :], in0=psos[c4],
                                            scalar=PV_UNSCALE, in1=rbc[:],
                                            op0=AluOpType.mult, op1=AluOpType.mult)
                                # spread the reciprocal sub-ops through the
                                # PV(c4=0) stretch so the DVE drains them
                                # between denominator adds; all four must
                                # precede the jj==15 op, whose closure also
                                # issues the oT mul that reads rbc in full
                                if c4 == 0 and jj % 4 == 2:
                                    ops.append(recip_ops[jj // 4])
                                ops.append(op_pv)

                        for oc in range(NT):
                            def op_u(oc=oc):
                                psU = psOp.tile([128, IB], F32, tag="psO")
                                for cc in range(NC2):
                                    nc.tensor.matmul(psU[:],
                                                     owT8[:, 2 * cc:2 * cc + 2, oc * 128:(oc + 1) * 128],
                                                     oT8[:, 2 * cc:2 * cc + 2, :],
                                                     start=(cc == 0), stop=(cc == NC2 - 1),
                                                     perf_mode=DR)
                                # bias + residual on DVE: the scalar engine
                                # must stay pure-exp here, or these ops delay
                                # the exp tail that QK(n+1) needs for psS
                                # recycling
                                nc.vector.tensor_scalar(out=osb[:, oc, :], in0=psU[:],
                                                        scalar1=1.0 / (WSCALE * OSCALE),
                                                        scalar2=ob_eff[:, oc:oc + 1],
                                                        op0=AluOpType.mult, op1=AluOpType.add)
                                nc.vector.tensor_add(osb[:, oc, :], osb[:, oc, :], xres[:, oc, :])
                                nc.sync.dma_start(out=ov[:, oc, il], in_=osb[:, oc, :])
                            ops.append(op_u)
                        return ops

                    # pending ops are paced to exhaust around slot 30 of the
                    # 32-slot QK loop so the PE stays fed through the
                    # exp-throttled QK tail instead of bunching early
                    pending = []
                    for n in range(NIB):
                        il = slice(n * IB, (n + 1) * IB)
                        P8 = p8p.tile([128, JT, IB], F8)
                        acc = accp.tile([128, IB], BF16)
                        npend = len(pending)
                        drained = 0
                        for j in range(JT):
                            psS = psSp.tile([128, IB], F32, tag="psS")
                            for cc in range(NC2):
                                nc.tensor.matmul(psS[:],
                                                 kT8[:, 2 * cc:2 * cc + 2, j * 128:(j + 1) * 128],
                                                 q8[:, 2 * cc:2 * cc + 2, il],
                                                 start=(cc == 0), stop=(cc == NC2 - 1),
                                                 perf_mode=DR)
                            nc.scalar.activation(out=P8[:, j, :], in_=psS[:],
                                                 func=mybir.ActivationFunctionType.Exp,
                                                 scale=ATT_SCALE, bias=negshift[:])
                            if j == 0:
                                nc.vector.tensor_copy(acc[:], P8[:, 0, :])
                            else:
                                nc.vector.tensor_add(acc[:], acc[:], P8[:, j, :])
                            target = min(npend, (npend * (j + 1) + 29) // 30)
                            while drained < target:
                                pending.pop(0)()
                                drained += 1
                        while pending:
                            pending.pop(0)()
                        pending = make_tail(n, P8, acc)
                    while pending:
                        pending.pop(0)()

    _split_multi_waits(nc)
    return nc


_RUNNER_CACHE = {}


class _Runner:
    """Builds the Bass graph once, compiles it through PJRT (shard_map over
    the 8 axon NeuronCores), and allows repeated execution for timing."""

    def __init__(self, S):
        self.S = S
        self.nc = build_nc(S)
        bass2jax.install_neuronx_cc_hook()
        nc = self.nc
        partition_name = (
            nc.partition_id_tensor.name if nc.partition_id_tensor else None
        )
        in_names, out_names, out_avals, zero_outs = [], [], [], []
        for alloc in nc.m.functions[0].allocations:
            if not isinstance(alloc, mybir.MemoryLocationSet):
                continue
            name = alloc.memorylocations[0].name
            if alloc.kind == "ExternalInput":
                if name != partition_name:
                    in_names.append(name)
            elif alloc.kind == "ExternalOutput":
                out_names.append(name)
                shape = tuple(alloc.tensor_shape)
                dtype = mybir.dt.np(alloc.dtype)
                out_avals.append(jax.core.ShapedArray(shape, dtype))
                zero_outs.append(np.zeros(shape, dtype))
        self.in_names = list(in_names)
        self.out_names = out_names
        self.out_avals = out_avals
        self.zero_outs = zero_outs
        all_in_names = in_names + out_names
        if partition_name is not None:
            all_in_names = all_in_names + [partition_name]

        def _body(*args):
            operands = list(args)
            if partition_name is not None:
                operands.append(bass2jax.partition_id_tensor())
            outs = bass2jax._bass_exec_p.bind(
                *operands,
                out_avals=tuple(out_avals),
                in_names=tuple(all_in_names),
                out_names=tuple(out_names),
                lowering_input_output_aliases=(),
                sim_require_finite=True,
                sim_require_nnan=True,
                nc=nc,
            )
            return tuple(outs)

        devices = jax.devices()[:8]
        self.mesh = Mesh(np.asarray(devices), ("core",))
        n_in = len(in_names) + len(out_names)
        self._fn = jax.jit(
            shard_map(
                _body, mesh=self.mesh,
                in_specs=(PartitionSpec("core"),) * n_in,
                out_specs=(PartitionSpec("core"),) * len(out_names),
                check_rep=False,
            )
        )

    def prepare(self, in_maps):
        sharding = NamedSharding(self.mesh, PartitionSpec("core"))
        concat = []
        for name in self.in_names:
            concat.append(np.concatenate([np.asarray(m[name]) for m in in_maps], axis=0))
        for z in self.zero_outs:
            concat.append(np.zeros((8 * z.shape[0], *z.shape[1:]), z.dtype))
        return [jax.device_put(a, sharding) for a in concat]

    def run(self, dev_args):
        return self._fn(*dev_args)


def _get_runner(S):
    if S not in _RUNNER_CACHE:
        _RUNNER_CACHE[S] = _Runner(S)
    return _RUNNER_CACHE[S]


def make_in_maps(x, gn_weight, gn_bias, qkv_w, qkv_b, out_w, out_b):
    b, c, h, w = x.shape
    S = h * w
    in_maps = []
    shared = {
        "gn_weight": np.ascontiguousarray(gn_weight, dtype=np.float32),
        "gn_bias": np.ascontiguousarray(gn_bias, dtype=np.float32),
        "qkv_w": np.ascontiguousarray(qkv_w, dtype=np.float32),
        "qkv_b": np.ascontiguousarray(qkv_b, dtype=np.float32),
        "out_w": np.ascontiguousarray(out_w, dtype=np.float32),
        "out_b": np.ascontiguousarray(out_b, dtype=np.float32),
    }
    for i in range(b):
        m = dict(shared)
        m["x"] = np.ascontiguousarray(np.asarray(x)[i].reshape(c, S), dtype=np.float32)
        in_maps.append(m)
    return in_maps


def kernel(x, gn_weight, gn_bias, qkv_w, qkv_b, out_w, out_b):
    x = np.asarray(x)
    b, c, h, w = x.shape
    assert b == 8 and c == C
    S = h * w
    r = _get_runner(S)
    in_maps = make_in_maps(x, gn_weight, gn_bias, qkv_w, qkv_b, out_w, out_b)
    outs = r.run(r.prepare(in_maps))
    idx = r.out_names.index("out")
    arr = np.asarray(outs[idx]).reshape(b, c, h, w)
    return arr.astype(np.float32)


# revision 28
# speedup vs baseline: 1.0367x; 1.0367x over previous
"""Trainium2 Bass kernel for nn_Attention_29661044146348.

Diffusion-style attention block: GroupNorm(32) -> 1x1-conv qkv -> single-head
attention over h*w positions (d = C = 512) -> 1x1-conv out -> residual.
Input x is [8, 512, 64, 64]; batch is data-parallel across the 8 NeuronCores
(one batch element per core), no collectives.

Per-core layout strategy ("S^T layout" flash attention, zero transposes in the
hot path):
  - scores are computed transposed, S^T[j, i] (keys on partitions), via
    lhsT = K^T chunks, rhs = Q^T block -- both natural outputs of the qkv
    matmul.
  - all hot-path matmuls run in fp8e4 with perf_mode=DoubleRow (two
    128-contraction planes per pass, ~1.5x bf16 PE rate). Value ranges are
    pre-scaled into fp8e4's safe window (TRN e4m3 max-normal is 240):
      * weights are stored as fp8(16*W); projections un-scale by 1/16 when
        leaving PSUM.
      * x is normalized on the fly: xn8 = fp8(A*x + B) via one scalar-engine
        activation per channel tile (A, B are the folded GroupNorm per-channel
        scale/bias, computed per 128-channel tile so the pipeline starts as
        soon as that tile's stats are in).
      * P = exp(scale*S^T - 2): the -2 shift keeps the max ~e^3.5 << 240 and
        cancels exactly in the softmax normalization.
      * O is normalized with rbc = 64/denom so fp8 O sits at ~N(0, 1.7),
        clear of the subnormal floor; the final projection un-scales by
        1/(16*64).
  - softmax denominators: bf16 elementwise accumulation on DVE, one
    (1/64)*ones[128x128] matmul for the cross-partition reduce+broadcast,
    and a DVE reciprocal for rbc.
  - the attention i-block loop is software-pipelined: block n-1's PV /
    denominator / output-projection matmuls are interleaved into block n's
    QK issue stream so the PE never waits for the scalar engine's exp.
  - the residual is streamed back from DRAM in fp32 per block (cheaper than
    holding a bf16 copy of x in SBUF and frees the GpSimd/DVE shared port).
  - engine balance: Scalar does xn8 + exp + K bias + half the V copies +
    final bias; DVE does denom adds + Q bias + other half of V + O normalize
    + residual; PE does matmuls + transposes only.
"""

import jax
import numpy as np
from jax.experimental.shard_map import shard_map
from jax.sharding import Mesh, NamedSharding, PartitionSpec

import bass_rust
import concourse.bass as bass
import concourse.tile as tile
from concourse import bass2jax, mybir
from concourse.alu_op_type import AluOpType
from concourse.masks import make_identity

F32 = mybir.dt.float32
BF16 = mybir.dt.bfloat16
F8 = mybir.dt.float8e4
DR = mybir.MatmulPerfMode.DoubleRow
IDENT_F = mybir.ActivationFunctionType.Identity

C = 512          # channels == attention dim
NT = C // 128    # channel tiles (4)
NC2 = NT // 2    # DoubleRow channel-tile pairs (2)
GROUPS = 32
EPS = 1e-5
ATT_SCALE = float(C) ** -0.5
IB = 512         # attention i-block (queries per block)
ESHIFT = 2.0     # exp(scale*s - ESHIFT); cancels in softmax normalization
WSCALE = 16.0    # fp8 weights stored as 16*W
OSCALE = 64.0    # fp8 O stored as 64*O/denom


def _split_multi_waits(nc):
    """The staged walrus build rejects >1 sync-wait per instruction; hoist
    extra waits onto single-wait NOPs placed immediately before."""
    ctr = 0
    for bb in nc.main_func.blocks:
        insts = bb.instructions
        i = 0
        while i < len(insts):
            ins = insts[i]
            si = ins.sync_info
            if si is not None:
                waits = list(si.on_wait)
                if len(waits) > 1:
                    si.on_wait = waits[-1:]
                    for w in waits[:-1]:
                        nop = mybir.InstNoOp(name=f"waitsplit-{ctr}", ins=[], outs=[])
                        ctr += 1
                        nop.engine = ins.engine
                        nop.sync_info = bass_rust.SyncInfo(on_wait=[w], on_update=[])
                        nc.register_instruction(nop, overwrite=True)
                        insts.insert(i, nop)
                        i += 1
            i += 1
    return ctr


def build_nc(S):
    S8 = S // 512     # seq chunks of 512
    JT = S // 128     # attention key chunks
    NIB = S // IB     # attention query blocks

    nc = bass.Bass()
    x_ext = nc.declare_dram_parameter("x", [C, S], F32, isOutput=False)
    gnw_ext = nc.declare_dram_parameter("gn_weight", [C], F32, isOutput=False)
    gnb_ext = nc.declare_dram_parameter("gn_bias", [C], F32, isOutput=False)
    qkvw_ext = nc.declare_dram_parameter("qkv_w", [3 * C, C], F32, isOutput=False)
    qkvb_ext = nc.declare_dram_parameter("qkv_b", [3 * C], F32, isOutput=False)
    outw_ext = nc.declare_dram_parameter("out_w", [C, C], F32, isOutput=False)
    outb_ext = nc.declare_dram_parameter("out_b", [C], F32, isOutput=False)
    out_ext = nc.declare_dram_parameter("out", [C, S], F32, isOutput=True)

    ov = out_ext[:].rearrange("(t p) s -> p t s", p=128)
    xv = x_ext[:].rearrange("(t p) s -> p t s", p=128)

    with tile.TileContext(nc) as tc:
        with (
            tc.tile_pool(name="consts", bufs=1) as consts,
            tc.tile_pool(name="big", bufs=1) as big,
            tc.tile_pool(name="gn_small", bufs=1) as gn_small,
        ):
            # ---------------- on-chip constants (no DMA) ----------------
            ident = consts.tile([128, 128], F32)
            make_identity(nc, ident)
            # 1/64 so the denominator reduce+broadcast matmul lands D/OSCALE
            # in PSUM; the plain reciprocal then directly yields OSCALE/D.
            onesbb = consts.tile([128, 128], BF16)
            nc.vector.memset(onesbb, 1.0 / OSCALE)
            ind = consts.tile([128, 8], F32)       # ind[p,g] = (p//16 == g)
            nc.vector.memset(ind, 1.0)
            nc.gpsimd.affine_select(
                out=ind, in_=ind, compare_op=mybir.AluOpType.is_ge, fill=0.0,
                base=0, pattern=[[-16, 8]], channel_multiplier=1)
            nc.gpsimd.affine_select(
                out=ind, in_=ind, compare_op=mybir.AluOpType.is_ge, fill=0.0,
                base=15, pattern=[[16, 8]], channel_multiplier=-1)
            indT = consts.tile([8, 128], F32)
            nc.vector.memset(indT, 1.0)
            nc.gpsimd.affine_select(
                out=indT, in_=indT, compare_op=mybir.AluOpType.is_ge, fill=0.0,
                base=0, pattern=[[1, 128]], channel_multiplier=-16)
            nc.gpsimd.affine_select(
                out=indT, in_=indT, compare_op=mybir.AluOpType.is_ge, fill=0.0,
                base=15, pattern=[[-1, 128]], channel_multiplier=16)
            eps8 = consts.tile([8, 1], F32)
            nc.vector.memset(eps8, EPS)
            negshift = consts.tile([128, 1], F32)
            nc.vector.memset(negshift, -ESHIFT)

            # resident big tiles (all fp8: 72KB/partition total)
            xn8 = big.tile([128, NT, S], F8)         # fp8(A*x + B) qkv input
            wT8 = big.tile([128, NT, 3 * C], F8)     # fp8(16 * qkv_w^T)
            owT8 = big.tile([128, NT, C], F8)        # fp8(16 * out_w^T)
            kT8 = big.tile([128, NT, S], F8)         # K^T  [c, s]
            q8 = big.tile([128, NT, S], F8)          # Q^T  [c, s] (resident)
            Vt8 = big.tile([128, JT, C], F8)         # V    [s, c] by key chunk

            qkvb12 = consts.tile([128, 3 * NT], F32)
            wv = consts.tile([128, NT], F32)
            bv = consts.tile([128, NT], F32)
            obt = consts.tile([128, NT], F32)
            ob_eff = consts.tile([128, NT], F32)
            gAt = gn_small.tile([128, NT], F32)      # per-channel GN scale
            gBt = gn_small.tile([128, NT], F32)      # per-channel GN bias
            stAll = gn_small.tile([128, NT, S8, 6], F32)
            vbr8 = gn_small.tile([128, NT, 2], F8)

            qb = qkvb12[:, 0:NT]
            kb = qkvb12[:, NT:2 * NT]

            # ------- startup: x stats + on-the-fly normalization ------------
            # Engine queues are kept decoupled: DVE runs stats + tiny combine
            # ops only (per channel tile, so tile t's xn8 can start as soon as
            # its own stats are in); Scalar runs the xn8 casts; the PE runs
            # the tiny combine matmuls, then weight transposes.
            with (
                tc.tile_pool(name="wnat", bufs=16) as wnat,
                tc.tile_pool(name="xstream", bufs=1) as xstream,
                tc.tile_pool(name="stp", bufs=2) as stp,
                tc.tile_pool(name="pst", bufs=3, space="PSUM") as pst,
                tc.tile_pool(name="psg", bufs=2, space="PSUM") as psg,
            ):
                # Weight DMAs lead: their transposes + fp8 casts run on the
                # PE/DVE/Scalar while the (larger) x stream is still in
                # flight. x follows as 4 chunk-DMAs per channel tile, t-major
                # so earlier tiles complete earlier and the per-tile
                # stats -> combine -> xn8 chains pipeline against the stream.
                wn_tiles = []
                for r in range(3 * C // 128):
                    wn = wnat.tile([128, C], F32)
                    nc.sync.dma_start(out=wn[:], in_=qkvw_ext[r * 128:(r + 1) * 128, :])
                    wn_tiles.append(wn)
                for r in range(C // 128):
                    wn = wnat.tile([128, C], F32)
                    nc.sync.dma_start(out=wn[:], in_=outw_ext[r * 128:(r + 1) * 128, :])
                    wn_tiles.append(wn)
                nc.sync.dma_start(out=qkvb12[:], in_=qkvb_ext[:].rearrange("(t p) -> p t", p=128))
                nc.sync.dma_start(out=wv[:], in_=gnw_ext[:].rearrange("(t p) -> p t", p=128))
                nc.sync.dma_start(out=bv[:], in_=gnb_ext[:].rearrange("(t p) -> p t", p=128))
                nc.sync.dma_start(out=obt[:], in_=outb_ext[:].rearrange("(t p) -> p t", p=128))
                xcs = [xstream.tile([128, S], F32, name=f"xc{t}") for t in range(NT)]
                XH = S // 8
                for t in range(NT):
                    for h in range(8):
                        nc.sync.dma_start(
                            out=xcs[t][:, h * XH:(h + 1) * XH],
                            in_=x_ext[t * 128:(t + 1) * 128, h * XH:(h + 1) * XH])

                # weight transposes (PE) -> fp8(16*W) casts, alternating
                # DVE/Scalar, K rows first (they gate the first qkv matmul)
                def wemit(r, c4, dst):
                    psT = pst.tile([128, 128], F32)
                    nc.tensor.transpose(psT[:], wn_tiles[r][:, c4 * 128:(c4 + 1) * 128], ident[:])
                    if (r + c4) % 2 == 0:
                        nc.vector.tensor_scalar_mul(dst, psT[:], WSCALE)
                    else:
                        nc.scalar.activation(out=dst, in_=psT[:], func=IDENT_F,
                                             scale=WSCALE)

                for r in list(range(4, 8)) + list(range(4)) + list(range(8, 12)):
                    for c4 in range(NT):
                        wemit(r, c4, wT8[:, c4, r * 128:(r + 1) * 128])
                for r in range(C // 128):
                    for c4 in range(NT):
                        wemit(12 + r, c4, owT8[:, c4, r * 128:(r + 1) * 128])

                for t in range(NT):
                    xc = xcs[t]
                    for s8 in range(S8):
                        nc.vector.bn_stats(out=stAll[:, t, s8, :],
                                           in_=xc[:, s8 * 512:(s8 + 1) * 512])
                    # per-tile GroupNorm combine (groups are 16 consecutive
                    # channels, so a 128-channel tile is self-contained)
                    mvt = stp.tile([128, 2], F32)
                    nc.vector.bn_aggr(out=mvt[:], in_=stAll[:, t, :, :])
                    st2 = stp.tile([128, 2], F32)
                    nc.vector.tensor_copy(st2[:, 0:1], mvt[:, 0:1])
                    sqt = stp.tile([128, 1], F32)
                    nc.vector.tensor_mul(sqt[:], mvt[:, 0:1], mvt[:, 0:1])
                    nc.vector.tensor_add(st2[:, 1:2], mvt[:, 1:2], sqt[:])
                    psG = psg.tile([8, 2], F32, tag="psG")
                    nc.tensor.matmul(psG[:], ind[:], st2[:], start=True, stop=True)
                    gsb = stp.tile([8, 2], F32)
                    nc.vector.tensor_scalar_mul(gsb[:], psG[:], 1.0 / 16.0)
                    sq8 = stp.tile([8, 1], F32)
                    nc.vector.tensor_mul(sq8[:], gsb[:, 0:1], gsb[:, 0:1])
                    varr = stp.tile([8, 1], F32)
                    nc.vector.tensor_sub(varr[:], gsb[:, 1:2], sq8[:])
                    sd8 = stp.tile([8, 1], F32)
                    nc.scalar.activation(out=sd8[:], in_=varr[:],
                                         func=mybir.ActivationFunctionType.Sqrt,
                                         bias=eps8[:], scale=1.0)
                    nc.vector.reciprocal(gsb[:, 1:2], sd8[:])
                    psBC = psg.tile([128, 2], F32, tag="psBC")
                    nc.tensor.matmul(psBC[:], indT[:], gsb[:], start=True, stop=True)
                    nc.vector.tensor_mul(gAt[:, t:t + 1], psBC[:, 1:2], wv[:, t:t + 1])
                    tmp1 = stp.tile([128, 1], F32)
                    nc.vector.tensor_mul(tmp1[:], psBC[:, 0:1], gAt[:, t:t + 1])
                    nc.vector.tensor_sub(gBt[:, t:t + 1], bv[:, t:t + 1], tmp1[:])
                    # xn8 = fp8(A*x + B), one scalar op per tile, pipelined
                    # against the x stream tile by tile
                    nc.scalar.activation(out=xn8[:, t, :], in_=xc[:], func=IDENT_F,
                                         bias=gBt[:, t:t + 1], scale=gAt[:, t:t + 1])

                # effective out bias: out_b + out_w @ v_bias (att rows sum
                # to 1, so the v bias is folded past the attention)
                nc.vector.memset(vbr8[:], 0.0)
                for c4 in range(NT):
                    nc.vector.tensor_copy(vbr8[:, c4, 0:1], qkvb12[:, 2 * NT + c4:2 * NT + c4 + 1])
                for oc in range(NT):
                    psE = psg.tile([128, 2], F32, tag="psBC")
                    for c4 in range(NT):
                        nc.tensor.matmul(psE[:], owT8[:, c4, oc * 128:(oc + 1) * 128],
                                         vbr8[:, c4, :],
                                         start=(c4 == 0), stop=(c4 == NT - 1))
                    nc.vector.tensor_scalar(out=ob_eff[:, oc:oc + 1], in0=psE[:, 0:1],
                                            scalar1=1.0 / WSCALE,
                                            scalar2=obt[:, oc:oc + 1],
                                            op0=AluOpType.mult, op1=AluOpType.add)

            # ---------------- qkv projection (fp8 DoubleRow) ----------------
            with (
                tc.tile_pool(name="p8p", bufs=2) as p8p,
                tc.tile_pool(name="accp", bufs=2) as accp,
                tc.tile_pool(name="rbcp", bufs=2) as rbcp,
                tc.tile_pool(name="oTp", bufs=2) as oTp,
                tc.tile_pool(name="osbp", bufs=2) as osbp,
                tc.tile_pool(name="xrp", bufs=2) as xrp,
            ):
                with tc.tile_pool(name="psq", bufs=4, space="PSUM") as psq:
                    for s8 in range(S8):
                        sl = slice(s8 * 512, (s8 + 1) * 512)
                        # K^T (scalar applies bias + 1/16)
                        for o4 in range(NT):
                            psK = psq.tile([128, 512], F32, tag="psq")
                            for cc in range(NC2):
                                nc.tensor.matmul(psK[:],
                                                 wT8[:, 2 * cc:2 * cc + 2, C + o4 * 128:C + (o4 + 1) * 128],
                                                 xn8[:, 2 * cc:2 * cc + 2, sl],
                                                 start=(cc == 0), stop=(cc == NC2 - 1),
                                                 perf_mode=DR)
                            nc.scalar.activation(out=kT8[:, o4, sl], in_=psK[:],
                                                 func=IDENT_F,
                                                 bias=kb[:, o4:o4 + 1], scale=1.0 / WSCALE)
                        # Q^T (DVE applies bias + 1/16), resident in SBUF
                        for o4 in range(NT):
                            psQ = psq.tile([128, 512], F32, tag="psq")
                            for cc in range(NC2):
                                nc.tensor.matmul(psQ[:],
                                                 wT8[:, 2 * cc:2 * cc + 2, o4 * 128:(o4 + 1) * 128],
                                                 xn8[:, 2 * cc:2 * cc + 2, sl],
                                                 start=(cc == 0), stop=(cc == NC2 - 1),
                                                 perf_mode=DR)
                            nc.vector.tensor_scalar(out=q8[:, o4, sl], in0=psQ[:],
                                                    scalar1=1.0 / WSCALE,
                                                    scalar2=qb[:, o4:o4 + 1],
                                                    op0=AluOpType.mult, op1=AluOpType.add)
                        # V (keys on partitions): lhsT = xn chunk, rhs = w_v^T.
                        # The 1/16 un-scale is folded into the PV normalize
                        # (rbc), so V extraction is a plain copy, alternating
                        # Scalar/DVE to balance the engines.
                        for j4 in range(4):
                            psV = psq.tile([128, 512], F32, tag="psq")
                            for cc in range(NC2):
                                nc.tensor.matmul(psV[:],
                                                 xn8[:, 2 * cc:2 * cc + 2,
                                                     s8 * 512 + j4 * 128:s8 * 512 + (j4 + 1) * 128],
                                                 wT8[:, 2 * cc:2 * cc + 2, 2 * C:3 * C],
                                                 start=(cc == 0), stop=(cc == NC2 - 1),
                                                 perf_mode=DR)
                            if j4 % 2 == 0:
                                nc.scalar.copy(Vt8[:, s8 * 4 + j4, :], psV[:])
                            else:
                                nc.vector.tensor_copy(Vt8[:, s8 * 4 + j4, :], psV[:])

                # ---------------- attention + output projection -------------
                # Software-pipelined: block n-1's PV + denominator + output
                # projection are interleaved into block n's QK issue stream so
                # the PE never stalls on the scalar engine's exp.
                with (
                    tc.tile_pool(name="psS", bufs=4, space="PSUM") as psSp,
                    tc.tile_pool(name="psO", bufs=3, space="PSUM") as psOp,
                    tc.tile_pool(name="psB", bufs=1, space="PSUM") as psBp,
                ):
                    # V carries a stray 16x (weights were 16*W and V skipped
                    # the un-scale): fold 1/16 into the O normalization.
                    PV_UNSCALE = 1.0 / WSCALE

                    def make_tail(n, P8, acc):
                        il = slice(n * IB, (n + 1) * IB)
                        oT8 = oTp.tile([128, NT, IB], F8)
                        rbc = rbcp.tile([128, IB], F32)
                        osb = osbp.tile([128, NT, IB], F32)
                        xres = xrp.tile([128, NT, IB], F32)
                        nc.sync.dma_start(out=xres[:], in_=xv[:, :, il])
                        psos = {}
                        ops = []

                        # the reciprocal is split into 4 sub-ops so it never
                        # clogs the DVE queue (a solid 3.4us reciprocal stalls
                        # the denominator adds and, through them, the PE)
                        psBbox = {}

                        def op_denom():
                            psBbox[0] = psBp.tile([128, IB], F32, tag="psB",
                                                  name=f"psB_{n}")
                            nc.tensor.matmul(psBbox[0][:], onesbb[:], acc[:],
                                             start=True, stop=True)
                        ops.append(op_denom)
                        recip_ops = []
                        for k in range(4):
                            def op_recip(k=k):
                                nc.vector.reciprocal(
                                    out=rbc[:, k * 128:(k + 1) * 128],
                                    in_=psBbox[0][:, k * 128:(k + 1) * 128])
                            recip_ops.append(op_recip)

                        for c4 in range(NT):
                            for jj in range(JT // 2):
                                def op_pv(c4=c4, jj=jj):
                                    if jj == 0:
                                        psos[c4] = psOp.tile([128, IB], F32, tag="psO",
                                                             name=f"psO_{n}_{c4}")
                                    nc.tensor.matmul(psos[c4],
                                                     Vt8[:, 2 * jj:2 * jj + 2, c4 * 128:(c4 + 1) * 128],
                                                     P8[:, 2 * jj:2 * jj + 2, :],
                                                     start=(jj == 0), stop=(jj == JT // 2 - 1),
                                                     perf_mode=DR)
                                    if jj == JT // 2 - 1:
                                        # oT = (psO/16) * (64/D), folded: psO * rbc / 16
                                        nc.vector.scalar_tensor_tensor(
                                            out=oT8[:, c4, :], in0=psos[c4],
                                            scalar=PV_UNSCALE, in1=rbc[:],
                                            op0=AluOpType.mult, op1=AluOpType.mult)
                                # spread the reciprocal sub-ops through the
                                # PV(c4=0) stretch so the DVE drains them
                                # between denominator adds; all four must
                                # precede the jj==15 op, whose closure also
                                # issues the oT mul that reads rbc in full
                                if c4 == 0 and jj % 4 == 2:
                                    ops.append(recip_ops[jj // 4])
                                ops.append(op_pv)

                        for oc in range(NT):
                            def op_u(oc=oc):
                                psU = psOp.tile([128, IB], F32, tag="psO")
                                for cc in range(NC2):
                                    nc.tensor.matmul(psU[:],
                                                     owT8[:, 2 * cc:2 * cc + 2, oc * 128:(oc + 1) * 128],
                                                     oT8[:, 2 * cc:2 * cc + 2, :],
                                                     start=(cc == 0), stop=(cc == NC2 - 1),
                                                     perf_mode=DR)
                                nc.scalar.activation(out=osb[:, oc, :], in_=psU[:],
                                                     func=IDENT_F,
                                                     bias=ob_eff[:, oc:oc + 1],
                                                     scale=1.0 / (WSCALE * OSCALE))
                                nc.vector.tensor_add(osb[:, oc, :], osb[:, oc, :], xres[:, oc, :])
                                nc.sync.dma_start(out=ov[:, oc, il], in_=osb[:, oc, :])
                            ops.append(op_u)
                        return ops

                    # pending ops are paced to exhaust around slot 30 of the
                    # 32-slot QK loop so the PE stays fed through the
                    # exp-throttled QK tail instead of bunching early
                    pending = []
                    for n in range(NIB):
                        il = slice(n * IB, (n + 1) * IB)
                        P8 = p8p.tile([128, JT, IB], F8)
                        acc = accp.tile([128, IB], BF16)
                        npend = len(pending)
                        drained = 0
                        for j in range(JT):
                            psS = psSp.tile([128, IB], F32, tag="psS")
                            for cc in range(NC2):
                                nc.tensor.matmul(psS[:],
                                                 kT8[:, 2 * cc:2 * cc + 2, j * 128:(j + 1) * 128],
                                                 q8[:, 2 * cc:2 * cc + 2, il],
                                                 start=(cc == 0), stop=(cc == NC2 - 1),
                                                 perf_mode=DR)
                            nc.scalar.activation(out=P8[:, j, :], in_=psS[:],
                                                 func=mybir.ActivationFunctionType.Exp,
                                                 scale=ATT_SCALE, bias=negshift[:])
                            if j == 0:
                                nc.vector.tensor_copy(acc[:], P8[:, 0, :])
                            else:
                                nc.vector.tensor_add(acc[:], acc[:], P8[:, j, :])
                            target = min(npend, (npend * (j + 1) + 29) // 30)
                            while drained < target:
                                pending.pop(0)()
                                drained += 1
                        while pending:
                            pending.pop(0)()
                        pending = make_tail(n, P8, acc)
                    while pending:
                        pending.pop(0)()

    _split_multi_waits(nc)
    return nc


_RUNNER_CACHE = {}


class _Runner:
    """Builds the Bass graph once, compiles it through PJRT (shard_map over
    the 8 axon NeuronCores), and allows repeated execution for timing."""

    def __init__(self, S):
        self.S = S
        self.nc = build_nc(S)
        bass2jax.install_neuronx_cc_hook()
        nc = self.nc
        partition_name = (
            nc.partition_id_tensor.name if nc.partition_id_tensor else None
        )
        in_names, out_names, out_avals, zero_outs = [], [], [], []
        for alloc in nc.m.functions[0].allocations:
            if not isinstance(alloc, mybir.MemoryLocationSet):
                continue
            name = alloc.memorylocations[0].name
            if alloc.kind == "ExternalInput":
                if name != partition_name:
                    in_names.append(name)
            elif alloc.kind == "ExternalOutput":
                out_names.append(name)
                shape = tuple(alloc.tensor_shape)
                dtype = mybir.dt.np(alloc.dtype)
                out_avals.append(jax.core.ShapedArray(shape, dtype))
                zero_outs.append(np.zeros(shape, dtype))
        self.in_names = list(in_names)
        self.out_names = out_names
        self.out_avals = out_avals
        self.zero_outs = zero_outs
        all_in_names = in_names + out_names
        if partition_name is not None:
            all_in_names = all_in_names + [partition_name]

        def _body(*args):
            operands = list(args)
            if partition_name is not None:
                operands.append(bass2jax.partition_id_tensor())
            outs = bass2jax._bass_exec_p.bind(
                *operands,
                out_avals=tuple(out_avals),
                in_names=tuple(all_in_names),
                out_names=tuple(out_names),
                lowering_input_output_aliases=(),
                sim_require_finite=True,
                sim_require_nnan=True,
                nc=nc,
            )
            return tuple(outs)

        devices = jax.devices()[:8]
        self.mesh = Mesh(np.asarray(devices), ("core",))
        n_in = len(in_names) + len(out_names)
        self._fn = jax.jit(
            shard_map(
                _body, mesh=self.mesh,
                in_specs=(PartitionSpec("core"),) * n_in,
                out_specs=(PartitionSpec("core"),) * len(out_names),
                check_rep=False,
            )
        )

    def prepare(self, in_maps):
        sharding = NamedSharding(self.mesh, PartitionSpec("core"))
        concat = []
        for name in self.in_names:
            concat.append(np.concatenate([np.asarray(m[name]) for m in in_maps], axis=0))
        for z in self.zero_outs:
            concat.append(np.zeros((8 * z.shape[0], *z.shape[1:]), z.dtype))
        return [jax.device_put(a, sharding) for a in concat]

    def run(self, dev_args):
        return self._fn(*dev_args)


def _get_runner(S):
    if S not in _RUNNER_CACHE:
        _RUNNER_CACHE[S] = _Runner(S)
    return _RUNNER_CACHE[S]


def make_in_maps(x, gn_weight, gn_bias, qkv_w, qkv_b, out_w, out_b):
    b, c, h, w = x.shape
    S = h * w
    in_maps = []
    shared = {
        "gn_weight": np.ascontiguousarray(gn_weight, dtype=np.float32),
        "gn_bias": np.ascontiguousarray(gn_bias, dtype=np.float32),
        "qkv_w": np.ascontiguousarray(qkv_w, dtype=np.float32),
        "qkv_b": np.ascontiguousarray(qkv_b, dtype=np.float32),
        "out_w": np.ascontiguousarray(out_w, dtype=np.float32),
        "out_b": np.ascontiguousarray(out_b, dtype=np.float32),
    }
    for i in range(b):
        m = dict(shared)
        m["x"] = np.ascontiguousarray(np.asarray(x)[i].reshape(c, S), dtype=np.float32)
        in_maps.append(m)
    return in_maps


def kernel(x, gn_weight, gn_bias, qkv_w, qkv_b, out_w, out_b):
    x = np.asarray(x)
    b, c, h, w = x.shape
    assert b == 8 and c == C
    S = h * w
    r = _get_runner(S)
    in_maps = make_in_maps(x, gn_weight, gn_bias, qkv_w, qkv_b, out_w, out_b)
    outs = r.run(r.prepare(in_maps))
    idx = r.out_names.index("out")
    arr = np.asarray(outs[idx]).reshape(b, c, h, w)
    return arr.astype(np.float32)


# revision 33
# speedup vs baseline: 1.0468x; 1.0097x over previous
"""Trainium2 Bass kernel for nn_Attention_29661044146348.

Diffusion-style attention block: GroupNorm(32) -> 1x1-conv qkv -> single-head
attention over h*w positions (d = C = 512) -> 1x1-conv out -> residual.
Input x is [8, 512, 64, 64]; batch is data-parallel across the 8 NeuronCores
(one batch element per core), no collectives.

Per-core layout strategy ("S^T layout" flash attention, zero transposes in the
hot path):
  - scores are computed transposed, S^T[j, i] (keys on partitions), via
    lhsT = K^T chunks, rhs = Q^T block -- both natural outputs of the qkv
    matmul.
  - all hot-path matmuls run in fp8e4 with perf_mode=DoubleRow (two
    128-contraction planes per pass, ~1.5x bf16 PE rate). Value ranges are
    pre-scaled into fp8e4's safe window (TRN e4m3 max-normal is 240):
      * weights are stored as fp8(16*W); projections un-scale by 1/16 when
        leaving PSUM.
      * x is normalized on the fly: xn8 = fp8(A*x + B) via one scalar-engine
        activation per channel tile (A, B are the folded GroupNorm per-channel
        scale/bias, computed per 128-channel tile so the pipeline starts as
        soon as that tile's stats are in).
      * P = exp(scale*S^T - 2): the -2 shift keeps the max ~e^3.5 << 240 and
        cancels exactly in the softmax normalization.
      * O is normalized with rbc = 64/denom so fp8 O sits at ~N(0, 1.7),
        clear of the subnormal floor; the final projection un-scales by
        1/(16*64).
  - softmax denominators: bf16 elementwise accumulation on DVE, one
    (1/64)*ones[128x128] matmul for the cross-partition reduce+broadcast,
    and a DVE reciprocal for rbc.
  - the attention i-block loop is software-pipelined: block n-1's PV /
    denominator / output-projection matmuls are interleaved into block n's
    QK issue stream so the PE never waits for the scalar engine's exp.
  - the residual is streamed back from DRAM in fp32 per block (cheaper than
    holding a bf16 copy of x in SBUF and frees the GpSimd/DVE shared port).
  - engine balance: Scalar does xn8 + exp + K bias + half the V copies +
    final bias; DVE does denom adds + Q bias + other half of V + O normalize
    + residual; PE does matmuls + transposes only.
"""

import jax
import numpy as np
from jax.experimental.shard_map import shard_map
from jax.sharding import Mesh, NamedSharding, PartitionSpec

import bass_rust
import concourse.bass as bass
import concourse.tile as tile
from concourse import bass2jax, mybir
from concourse.alu_op_type import AluOpType
from concourse.masks import make_identity

F32 = mybir.dt.float32
BF16 = mybir.dt.bfloat16
F8 = mybir.dt.float8e4
DR = mybir.MatmulPerfMode.DoubleRow
IDENT_F = mybir.ActivationFunctionType.Identity

C = 512          # channels == attention dim
NT = C // 128    # channel tiles (4)
NC2 = NT // 2    # DoubleRow channel-tile pairs (2)
GROUPS = 32
EPS = 1e-5
ATT_SCALE = float(C) ** -0.5
IB = 512         # attention i-block (queries per block)
ESHIFT = 2.0     # exp(scale*s - ESHIFT); cancels in softmax normalization
WSCALE = 16.0    # fp8 weights stored as 16*W
OSCALE = 64.0    # fp8 O stored as 64*O/denom


def _split_multi_waits(nc):
    """The staged walrus build rejects >1 sync-wait per instruction; hoist
    extra waits onto single-wait NOPs placed immediately before."""
    ctr = 0
    for bb in nc.main_func.blocks:
        insts = bb.instructions
        i = 0
        while i < len(insts):
            ins = insts[i]
            si = ins.sync_info
            if si is not None:
                waits = list(si.on_wait)
                if len(waits) > 1:
                    si.on_wait = waits[-1:]
                    for w in waits[:-1]:
                        nop = mybir.InstNoOp(name=f"waitsplit-{ctr}", ins=[], outs=[])
                        ctr += 1
                        nop.engine = ins.engine
                        nop.sync_info = bass_rust.SyncInfo(on_wait=[w], on_update=[])
                        nc.register_instruction(nop, overwrite=True)
                        insts.insert(i, nop)
                        i += 1
            i += 1
    return ctr


def build_nc(S):
    S8 = S // 512     # seq chunks of 512
    JT = S // 128     # attention key chunks
    NIB = S // IB     # attention query blocks

    nc = bass.Bass()
    x_ext = nc.declare_dram_parameter("x", [C, S], F32, isOutput=False)
    gnw_ext = nc.declare_dram_parameter("gn_weight", [C], F32, isOutput=False)
    gnb_ext = nc.declare_dram_parameter("gn_bias", [C], F32, isOutput=False)
    qkvw_ext = nc.declare_dram_parameter("qkv_w", [3 * C, C], F32, isOutput=False)
    qkvb_ext = nc.declare_dram_parameter("qkv_b", [3 * C], F32, isOutput=False)
    outw_ext = nc.declare_dram_parameter("out_w", [C, C], F32, isOutput=False)
    outb_ext = nc.declare_dram_parameter("out_b", [C], F32, isOutput=False)
    out_ext = nc.declare_dram_parameter("out", [C, S], F32, isOutput=True)

    ov = out_ext[:].rearrange("(t p) s -> p t s", p=128)
    xv = x_ext[:].rearrange("(t p) s -> p t s", p=128)

    with tile.TileContext(nc) as tc:
        with (
            tc.tile_pool(name="consts", bufs=1) as consts,
            tc.tile_pool(name="big", bufs=1) as big,
            tc.tile_pool(name="gn_small", bufs=1) as gn_small,
        ):
            # ---------------- on-chip constants (no DMA) ----------------
            ident = consts.tile([128, 128], F32)
            make_identity(nc, ident)
            # 1/64 so the denominator reduce+broadcast matmul lands D/OSCALE
            # in PSUM; the plain reciprocal then directly yields OSCALE/D.
            onesbb = consts.tile([128, 128], BF16)
            nc.vector.memset(onesbb, 1.0 / OSCALE)
            ind = consts.tile([128, 8], F32)       # ind[p,g] = (p//16 == g)
            nc.vector.memset(ind, 1.0)
            nc.gpsimd.affine_select(
                out=ind, in_=ind, compare_op=mybir.AluOpType.is_ge, fill=0.0,
                base=0, pattern=[[-16, 8]], channel_multiplier=1)
            nc.gpsimd.affine_select(
                out=ind, in_=ind, compare_op=mybir.AluOpType.is_ge, fill=0.0,
                base=15, pattern=[[16, 8]], channel_multiplier=-1)
            indT = consts.tile([8, 128], F32)
            nc.vector.memset(indT, 1.0)
            nc.gpsimd.affine_select(
                out=indT, in_=indT, compare_op=mybir.AluOpType.is_ge, fill=0.0,
                base=0, pattern=[[1, 128]], channel_multiplier=-16)
            nc.gpsimd.affine_select(
                out=indT, in_=indT, compare_op=mybir.AluOpType.is_ge, fill=0.0,
                base=15, pattern=[[-1, 128]], channel_multiplier=16)
            eps8 = consts.tile([8, 1], F32)
            nc.vector.memset(eps8, EPS)
            negshift = consts.tile([128, 1], F32)
            nc.vector.memset(negshift, -ESHIFT)

            # resident big tiles (all fp8: 72KB/partition total)
            xn8 = big.tile([128, NT, S], F8)         # fp8(A*x + B) qkv input
            wT8 = big.tile([128, NT, 3 * C], F8)     # fp8(16 * qkv_w^T)
            owT8 = big.tile([128, NT, C], F8)        # fp8(16 * out_w^T)
            kT8 = big.tile([128, NT, S], F8)         # K^T  [c, s]
            q8 = big.tile([128, NT, S], F8)          # Q^T  [c, s] (resident)
            Vt8 = big.tile([128, JT, C], F8)         # V    [s, c] by key chunk

            qkvb12 = consts.tile([128, 3 * NT], F32)
            wv = consts.tile([128, NT], F32)
            bv = consts.tile([128, NT], F32)
            obt = consts.tile([128, NT], F32)
            ob_eff = consts.tile([128, NT], F32)
            gAt = gn_small.tile([128, NT], F32)      # per-channel GN scale
            gBt = gn_small.tile([128, NT], F32)      # per-channel GN bias
            stAll = gn_small.tile([128, NT, S8, 6], F32)
            vbr8 = gn_small.tile([128, NT, 2], F8)

            qb = qkvb12[:, 0:NT]
            kb = qkvb12[:, NT:2 * NT]

            # ------- startup: x stats + on-the-fly normalization ------------
            # Engine queues are kept decoupled: DVE runs stats + tiny combine
            # ops only (per channel tile, so tile t's xn8 can start as soon as
            # its own stats are in); Scalar runs the xn8 casts; the PE runs
            # the tiny combine matmuls, then weight transposes.
            with (
                tc.tile_pool(name="wnat", bufs=16) as wnat,
                tc.tile_pool(name="xstream", bufs=1) as xstream,
                tc.tile_pool(name="stp", bufs=2) as stp,
                tc.tile_pool(name="pst", bufs=3, space="PSUM") as pst,
                tc.tile_pool(name="psg", bufs=2, space="PSUM") as psg,
            ):
                # Weight DMAs lead: their transposes + fp8 casts run on the
                # PE/DVE/Scalar while the (larger) x stream is still in
                # flight. x follows as 4 chunk-DMAs per channel tile, t-major
                # so earlier tiles complete earlier and the per-tile
                # stats -> combine -> xn8 chains pipeline against the stream.
                wn_tiles = []
                for r in range(3 * C // 128):
                    wn = wnat.tile([128, C], F32)
                    nc.sync.dma_start(out=wn[:], in_=qkvw_ext[r * 128:(r + 1) * 128, :])
                    wn_tiles.append(wn)
                for r in range(C // 128):
                    wn = wnat.tile([128, C], F32)
                    nc.sync.dma_start(out=wn[:], in_=outw_ext[r * 128:(r + 1) * 128, :])
                    wn_tiles.append(wn)
                nc.sync.dma_start(out=qkvb12[:], in_=qkvb_ext[:].rearrange("(t p) -> p t", p=128))
                nc.sync.dma_start(out=wv[:], in_=gnw_ext[:].rearrange("(t p) -> p t", p=128))
                nc.sync.dma_start(out=bv[:], in_=gnb_ext[:].rearrange("(t p) -> p t", p=128))
                nc.sync.dma_start(out=obt[:], in_=outb_ext[:].rearrange("(t p) -> p t", p=128))
                xcs = [xstream.tile([128, S], F32, name=f"xc{t}") for t in range(NT)]
                XH = S // 4
                for t in range(NT):
                    for h in range(4):
                        nc.sync.dma_start(
                            out=xcs[t][:, h * XH:(h + 1) * XH],
                            in_=x_ext[t * 128:(t + 1) * 128, h * XH:(h + 1) * XH])

                # weight transposes (PE) -> fp8(16*W) casts, alternating
                # DVE/Scalar, K rows first (they gate the first qkv matmul)
                def wemit(r, c4, dst):
                    psT = pst.tile([128, 128], F32)
                    nc.tensor.transpose(psT[:], wn_tiles[r][:, c4 * 128:(c4 + 1) * 128], ident[:])
                    if (r + c4) % 2 == 0:
                        nc.vector.tensor_scalar_mul(dst, psT[:], WSCALE)
                    else:
                        nc.scalar.activation(out=dst, in_=psT[:], func=IDENT_F,
                                             scale=WSCALE)

                for r in list(range(4, 8)) + list(range(4)) + list(range(8, 12)):
                    for c4 in range(NT):
                        wemit(r, c4, wT8[:, c4, r * 128:(r + 1) * 128])
                for r in range(C // 128):
                    for c4 in range(NT):
                        wemit(12 + r, c4, owT8[:, c4, r * 128:(r + 1) * 128])

                for t in range(NT):
                    xc = xcs[t]
                    for s8 in range(S8):
                        nc.vector.bn_stats(out=stAll[:, t, s8, :],
                                           in_=xc[:, s8 * 512:(s8 + 1) * 512])
                    # per-tile GroupNorm combine (groups are 16 consecutive
                    # channels, so a 128-channel tile is self-contained)
                    mvt = stp.tile([128, 2], F32)
                    nc.vector.bn_aggr(out=mvt[:], in_=stAll[:, t, :, :])
                    st2 = stp.tile([128, 2], F32)
                    nc.vector.tensor_copy(st2[:, 0:1], mvt[:, 0:1])
                    sqt = stp.tile([128, 1], F32)
                    nc.vector.tensor_mul(sqt[:], mvt[:, 0:1], mvt[:, 0:1])
                    nc.vector.tensor_add(st2[:, 1:2], mvt[:, 1:2], sqt[:])
                    psG = psg.tile([8, 2], F32, tag="psG")
                    nc.tensor.matmul(psG[:], ind[:], st2[:], start=True, stop=True)
                    gsb = stp.tile([8, 2], F32)
                    nc.vector.tensor_scalar_mul(gsb[:], psG[:], 1.0 / 16.0)
                    sq8 = stp.tile([8, 1], F32)
                    nc.vector.tensor_mul(sq8[:], gsb[:, 0:1], gsb[:, 0:1])
                    varr = stp.tile([8, 1], F32)
                    nc.vector.tensor_sub(varr[:], gsb[:, 1:2], sq8[:])
                    sd8 = stp.tile([8, 1], F32)
                    nc.scalar.activation(out=sd8[:], in_=varr[:],
                                         func=mybir.ActivationFunctionType.Sqrt,
                                         bias=eps8[:], scale=1.0)
                    nc.vector.reciprocal(gsb[:, 1:2], sd8[:])
                    psBC = psg.tile([128, 2], F32, tag="psBC")
                    nc.tensor.matmul(psBC[:], indT[:], gsb[:], start=True, stop=True)
                    nc.vector.tensor_mul(gAt[:, t:t + 1], psBC[:, 1:2], wv[:, t:t + 1])
                    tmp1 = stp.tile([128, 1], F32)
                    nc.vector.tensor_mul(tmp1[:], psBC[:, 0:1], gAt[:, t:t + 1])
                    nc.vector.tensor_sub(gBt[:, t:t + 1], bv[:, t:t + 1], tmp1[:])
                    # xn8 = fp8(A*x + B), one scalar op per tile, pipelined
                    # against the x stream tile by tile
                    nc.scalar.activation(out=xn8[:, t, :], in_=xc[:], func=IDENT_F,
                                         bias=gBt[:, t:t + 1], scale=gAt[:, t:t + 1])

                # effective out bias: out_b + out_w @ v_bias (att rows sum
                # to 1, so the v bias is folded past the attention)
                nc.vector.memset(vbr8[:], 0.0)
                for c4 in range(NT):
                    nc.vector.tensor_copy(vbr8[:, c4, 0:1], qkvb12[:, 2 * NT + c4:2 * NT + c4 + 1])
                for oc in range(NT):
                    psE = psg.tile([128, 2], F32, tag="psBC")
                    for c4 in range(NT):
                        nc.tensor.matmul(psE[:], owT8[:, c4, oc * 128:(oc + 1) * 128],
                                         vbr8[:, c4, :],
                                         start=(c4 == 0), stop=(c4 == NT - 1))
                    nc.vector.tensor_scalar(out=ob_eff[:, oc:oc + 1], in0=psE[:, 0:1],
                                            scalar1=1.0 / WSCALE,
                                            scalar2=obt[:, oc:oc + 1],
                                            op0=AluOpType.mult, op1=AluOpType.add)

            # ---------------- qkv projection (fp8 DoubleRow) ----------------
            with (
                tc.tile_pool(name="p8p", bufs=2) as p8p,
                tc.tile_pool(name="accp", bufs=2) as accp,
                tc.tile_pool(name="rbcp", bufs=2) as rbcp,
                tc.tile_pool(name="oTp", bufs=2) as oTp,
                tc.tile_pool(name="osbp", bufs=2) as osbp,
                tc.tile_pool(name="xrp", bufs=2) as xrp,
            ):
                with tc.tile_pool(name="psq", bufs=4, space="PSUM") as psq:
                    for s8 in range(S8):
                        sl = slice(s8 * 512, (s8 + 1) * 512)
                        # K^T (scalar applies bias + 1/16)
                        for o4 in range(NT):
                            psK = psq.tile([128, 512], F32, tag="psq")
                            for cc in range(NC2):
                                nc.tensor.matmul(psK[:],
                                                 wT8[:, 2 * cc:2 * cc + 2, C + o4 * 128:C + (o4 + 1) * 128],
                                                 xn8[:, 2 * cc:2 * cc + 2, sl],
                                                 start=(cc == 0), stop=(cc == NC2 - 1),
                                                 perf_mode=DR)
                            nc.scalar.activation(out=kT8[:, o4, sl], in_=psK[:],
                                                 func=IDENT_F,
                                                 bias=kb[:, o4:o4 + 1], scale=1.0 / WSCALE)
                        # Q^T (DVE applies bias + 1/16), resident in SBUF
                        for o4 in range(NT):
                            psQ = psq.tile([128, 512], F32, tag="psq")
                            for cc in range(NC2):
                                nc.tensor.matmul(psQ[:],
                                                 wT8[:, 2 * cc:2 * cc + 2, o4 * 128:(o4 + 1) * 128],
                                                 xn8[:, 2 * cc:2 * cc + 2, sl],
                                                 start=(cc == 0), stop=(cc == NC2 - 1),
                                                 perf_mode=DR)
                            nc.vector.tensor_scalar(out=q8[:, o4, sl], in0=psQ[:],
                                                    scalar1=1.0 / WSCALE,
                                                    scalar2=qb[:, o4:o4 + 1],
                                                    op0=AluOpType.mult, op1=AluOpType.add)
                        # V (keys on partitions): lhsT = xn chunk, rhs = w_v^T.
                        # The 1/16 un-scale is folded into the PV normalize
                        # (rbc), so V extraction is a plain copy, alternating
                        # Scalar/DVE to balance the engines.
                        for j4 in range(4):
                            psV = psq.tile([128, 512], F32, tag="psq")
                            for cc in range(NC2):
                                nc.tensor.matmul(psV[:],
                                                 xn8[:, 2 * cc:2 * cc + 2,
                                                     s8 * 512 + j4 * 128:s8 * 512 + (j4 + 1) * 128],
                                                 wT8[:, 2 * cc:2 * cc + 2, 2 * C:3 * C],
                                                 start=(cc == 0), stop=(cc == NC2 - 1),
                                                 perf_mode=DR)
                            if j4 % 2 == 0:
                                nc.scalar.copy(Vt8[:, s8 * 4 + j4, :], psV[:])
                            else:
                                nc.vector.tensor_copy(Vt8[:, s8 * 4 + j4, :], psV[:])

                # ---------------- attention + output projection -------------
                # Software-pipelined: block n-1's PV + denominator + output
                # projection are interleaved into block n's QK issue stream so
                # the PE never stalls on the scalar engine's exp.
                with (
                    tc.tile_pool(name="psS", bufs=4, space="PSUM") as psSp,
                    tc.tile_pool(name="psO", bufs=3, space="PSUM") as psOp,
                    tc.tile_pool(name="psB", bufs=1, space="PSUM") as psBp,
                ):
                    # V carries a stray 16x (weights were 16*W and V skipped
                    # the un-scale): fold 1/16 into the O normalization.
                    PV_UNSCALE = 1.0 / WSCALE

                    def make_tail(n, P8, acc):
                        il = slice(n * IB, (n + 1) * IB)
                        oT8 = oTp.tile([128, NT, IB], F8)
                        rbc = rbcp.tile([128, IB], F32)
                        osb = osbp.tile([128, NT, IB], F32)
                        xres = xrp.tile([128, NT, IB], F32)
                        nc.sync.dma_start(out=xres[:], in_=xv[:, :, il])
                        psos = {}
                        ops = []
                        uops = []

                        # the reciprocal is split into 4 sub-ops so it never
                        # clogs the DVE queue (a solid 3.4us reciprocal stalls
                        # the denominator adds and, through them, the PE)
                        psBbox = {}

                        def op_denom():
                            psBbox[0] = psBp.tile([128, IB], F32, tag="psB",
                                                  name=f"psB_{n}")
                            nc.tensor.matmul(psBbox[0][:], onesbb[:], acc[:],
                                             start=True, stop=True)
                        ops.append(op_denom)
                        recip_ops = []
                        for k in range(4):
                            def op_recip(k=k):
                                nc.vector.reciprocal(
                                    out=rbc[:, k * 128:(k + 1) * 128],
                                    in_=psBbox[0][:, k * 128:(k + 1) * 128])
                            recip_ops.append(op_recip)

                        for c4 in range(NT):
                            for jj in range(JT // 2):
                                def op_pv(c4=c4, jj=jj):
                                    if jj == 0:
                                        psos[c4] = psOp.tile([128, IB], F32, tag="psO",
                                                             name=f"psO_{n}_{c4}")
                                    nc.tensor.matmul(psos[c4],
                                                     Vt8[:, 2 * jj:2 * jj + 2, c4 * 128:(c4 + 1) * 128],
                                                     P8[:, 2 * jj:2 * jj + 2, :],
                                                     start=(jj == 0), stop=(jj == JT // 2 - 1),
                                                     perf_mode=DR)
                                    if jj == JT // 2 - 1:
                                        # oT = (psO/16) * (64/D), folded: psO * rbc / 16
                                        nc.vector.scalar_tensor_tensor(
                                            out=oT8[:, c4, :], in0=psos[c4],
                                            scalar=PV_UNSCALE, in1=rbc[:],
                                            op0=AluOpType.mult, op1=AluOpType.mult)
                                # spread the reciprocal sub-ops through the
                                # PV(c4=0) stretch so the DVE drains them
                                # between denominator adds; all four must
                                # precede the jj==15 op, whose closure also
                                # issues the oT mul that reads rbc in full
                                if c4 == 0 and jj % 4 == 2:
                                    ops.append(recip_ops[jj // 4])
                                ops.append(op_pv)

                        for oc in range(NT):
                            def op_u(oc=oc):
                                psU = psOp.tile([128, IB], F32, tag="psO")
                                for cc in range(NC2):
                                    nc.tensor.matmul(psU[:],
                                                     owT8[:, 2 * cc:2 * cc + 2, oc * 128:(oc + 1) * 128],
                                                     oT8[:, 2 * cc:2 * cc + 2, :],
                                                     start=(cc == 0), stop=(cc == NC2 - 1),
                                                     perf_mode=DR)
                                nc.scalar.activation(out=osb[:, oc, :], in_=psU[:],
                                                     func=IDENT_F,
                                                     bias=ob_eff[:, oc:oc + 1],
                                                     scale=1.0 / (WSCALE * OSCALE))
                                nc.vector.tensor_add(osb[:, oc, :], osb[:, oc, :], xres[:, oc, :])
                                nc.sync.dma_start(out=ov[:, oc, il], in_=osb[:, oc, :])
                            uops.append(op_u)
                        return ops, uops

                    # pending ops are paced to exhaust around slot 30 of the
                    # 32-slot QK loop so the PE stays fed through the
                    # exp-throttled QK tail instead of bunching early.
                    # Output-projection ops lag one extra block (uops of
                    # block n-2 drain at the head of block n): their scalar
                    # bias IDENTs then hit the exp queue's slack head, not
                    # the tail that QK(n+1)'s psS recycling depends on.
                    pending = []
                    uops_prev = []
                    for n in range(NIB):
                        il = slice(n * IB, (n + 1) * IB)
                        P8 = p8p.tile([128, JT, IB], F8)
                        acc = accp.tile([128, IB], BF16)
                        npend = len(pending)
                        drained = 0
                        for j in range(JT):
                            psS = psSp.tile([128, IB], F32, tag="psS")
                            for cc in range(NC2):
                                nc.tensor.matmul(psS[:],
                                                 kT8[:, 2 * cc:2 * cc + 2, j * 128:(j + 1) * 128],
                                                 q8[:, 2 * cc:2 * cc + 2, il],
                                                 start=(cc == 0), stop=(cc == NC2 - 1),
                                                 perf_mode=DR)
                            nc.scalar.activation(out=P8[:, j, :], in_=psS[:],
                                                 func=mybir.ActivationFunctionType.Exp,
                                                 scale=ATT_SCALE, bias=negshift[:])
                            if j == 0:
                                nc.vector.tensor_copy(acc[:], P8[:, 0, :])
                            else:
                                nc.vector.tensor_add(acc[:], acc[:], P8[:, j, :])
                            target = min(npend, (npend * (j + 1) + 29) // 30)
                            while drained < target:
                                pending.pop(0)()
                                drained += 1
                        while pending:
                            pending.pop(0)()
                        ops_n, uops_n = make_tail(n, P8, acc)
                        pending = uops_prev + ops_n
                        uops_prev = uops_n
                    while pending:
                        pending.pop(0)()
                    while uops_prev:
                        uops_prev.pop(0)()

    _split_multi_waits(nc)
    return nc


_RUNNER_CACHE = {}


class _Runner:
    """Builds the Bass graph once, compiles it through PJRT (shard_map over
    the 8 axon NeuronCores), and allows repeated execution for timing."""

    def __init__(self, S):
        self.S = S
        self.nc = build_nc(S)
        bass2jax.install_neuronx_cc_hook()
        nc = self.nc
        partition_name = (
            nc.partition_id_tensor.name if nc.partition_id_tensor else None
        )
        in_names, out_names, out_avals, zero_outs = [], [], [], []
        for alloc in nc.m.functions[0].allocations:
            if not isinstance(alloc, mybir.MemoryLocationSet):
                continue
            name = alloc.memorylocations[0].name
            if alloc.kind == "ExternalInput":
                if name != partition_name:
                    in_names.append(name)
            elif alloc.kind == "ExternalOutput":
                out_names.append(name)
                shape = tuple(alloc.tensor_shape)
                dtype = mybir.dt.np(alloc.dtype)
                out_avals.append(jax.core.ShapedArray(shape, dtype))
                zero_outs.append(np.zeros(shape, dtype))
        self.in_names = list(in_names)
        self.out_names = out_names
        self.out_avals = out_avals
        self.zero_outs = zero_outs
        all_in_names = in_names + out_names
        if partition_name is not None:
            all_in_names = all_in_names + [partition_name]

        def _body(*args):
            operands = list(args)
            if partition_name is not None:
                operands.append(bass2jax.partition_id_tensor())
            outs = bass2jax._bass_exec_p.bind(
                *operands,
                out_avals=tuple(out_avals),
                in_names=tuple(all_in_names),
                out_names=tuple(out_names),
                lowering_input_output_aliases=(),
                sim_require_finite=True,
                sim_require_nnan=True,
                nc=nc,
            )
            return tuple(outs)

        devices = jax.devices()[:8]
        self.mesh = Mesh(np.asarray(devices), ("core",))
        n_in = len(in_names) + len(out_names)
        self._fn = jax.jit(
            shard_map(
                _body, mesh=self.mesh,
                in_specs=(PartitionSpec("core"),) * n_in,
                out_specs=(PartitionSpec("core"),) * len(out_names),
                check_rep=False,
            )
        )

    def prepare(self, in_maps):
        sharding = NamedSharding(self.mesh, PartitionSpec("core"))
        concat = []
        for name in self.in_names:
            concat.append(np.concatenate([np.asarray(m[name]) for m in in_maps], axis=0))
        for z in self.zero_outs:
            concat.append(np.zeros((8 * z.shape[0], *z.shape[1:]), z.dtype))
        return [jax.device_put(a, sharding) for a in concat]

    def run(self, dev_args):
        return self._fn(*dev_args)


def _get_runner(S):
    if S not in _RUNNER_CACHE:
        _RUNNER_CACHE[S] = _Runner(S)
    return _RUNNER_CACHE[S]


def make_in_maps(x, gn_weight, gn_bias, qkv_w, qkv_b, out_w, out_b):
    b, c, h, w = x.shape
    S = h * w
    in_maps = []
    shared = {
        "gn_weight": np.ascontiguousarray(gn_weight, dtype=np.float32),
        "gn_bias": np.ascontiguousarray(gn_bias, dtype=np.float32),
        "qkv_w": np.ascontiguousarray(qkv_w, dtype=np.float32),
        "qkv_b": np.ascontiguousarray(qkv_b, dtype=np.float32),
        "out_w": np.ascontiguousarray(out_w, dtype=np.float32),
        "out_b": np.ascontiguousarray(out_b, dtype=np.float32),
    }
    for i in range(b):
        m = dict(shared)
        m["x"] = np.ascontiguousarray(np.asarray(x)[i].reshape(c, S), dtype=np.float32)
        in_maps.append(m)
    return in_maps


def kernel(x, gn_weight, gn_bias, qkv_w, qkv_b, out_w, out_b):
    x = np.asarray(x)
    b, c, h, w = x.shape
    assert b == 8 and c == C
    S = h * w
    r = _get_runner(S)
    in_maps = make_in_maps(x, gn_weight, gn_bias, qkv_w, qkv_b, out_w, out_b)
    outs = r.run(r.prepare(in_maps))
    idx = r.out_names.index("out")
    arr = np.asarray(outs[idx]).reshape(b, c, h, w)
    return arr.astype(np.float32)
